# revision 33
# baseline (speedup 1.0000x reference)
"""Trainium2 Bass kernel: DecorrelationNormalization (IterNorm whitening).

Input  x: (64, 56, 56, 256) f32, gamma/beta: (1,1,1,256) f32.
Sharding: data-parallel over batch across 8 NeuronCores (8 batches/core).
Per-group (4 groups of 64 channels) covariance stats are computed locally
as uncentered second moments + channel sums, packed into 66KB, all-reduced,
the tiny Newton-Schulz iteration is replicated on every core (bf16 matmuls),
and the whitening is applied channel-major with W stationary, writing bf16
output that the host transposes/upcasts.
"""

import sys

for p in ("/opt/trn_rl_repo", "/opt/pypackages"):
    if p not in sys.path:
        sys.path.append(p)

import numpy as np

import concourse.bass as bass
import concourse.bacc as bacc
import concourse.tile as tile
from concourse import mybir
from concourse.bass_utils import run_bass_kernel_spmd
F32 = mybir.dt.float32
BF16 = mybir.dt.bfloat16

# Problem constants (hardcoded per spec).
B, H, W, C = 64, 56, 56, 256
NCORES = 8
BLOC = B // NCORES                    # 8 batches per core
NLOC = BLOC * H * W                   # 25088 positions per core
NGLOB = B * H * W                     # 200704 positions globally
CHUNK = 128                           # positions per chunk (partition dim)
NCHUNK = NLOC // CHUNK                # 196
SUP = 14                              # chunks per super-tile (DMA batch)
NSUP = NCHUNK // SUP                  # 14, exact
POSW = 3584                           # pass-2 output window (positions)
NWIN = NLOC // POSW                   # 7, exact
MMW = 512                             # pass-2 matmul moving width
NMM = POSW // MMW                     # 7
EPS = 1e-5
ITER_NUM = 5

AOP = mybir.AluOpType
AFT = mybir.ActivationFunctionType


def build_bass() -> bass.Bass:
    nc = bacc.Bacc(None, num_devices=NCORES)

    x_d = nc.declare_dram_parameter("x", [BLOC, H, W, C], F32, isOutput=False)
    g_d = nc.declare_dram_parameter("gamma", [1, 1, 1, C], F32, isOutput=False)
    b_d = nc.declare_dram_parameter("beta", [1, 1, 1, C], F32, isOutput=False)
    eye_d = nc.declare_dram_parameter("eye", [128, 128], F32, isOutput=False)
    y_d = nc.declare_dram_parameter("out", [2, 128, NLOC], BF16, isOutput=True)

    xv = x_d[:].flatten_outer_dims()     # (25088, 256)
    gv = g_d[:].flatten_outer_dims()     # (1, 256)
    bv = b_d[:].flatten_outer_dims()     # (1, 256)

    with tile.TileContext(nc) as tc:
        with (
            tc.tile_pool(name="keep", bufs=1) as keep,
            tc.tile_pool(name="inp", bufs=3) as inp,
            tc.tile_pool(name="btp", bufs=3) as btp,
            tc.tile_pool(name="outp", bufs=3) as outp,
            tc.tile_pool(name="small", bufs=1) as small,
            tc.tile_pool(name="ps_acc", bufs=1, space="PSUM") as ps_acc,
            tc.tile_pool(name="ps_rot", bufs=2, space="PSUM") as ps_rot,
            tc.tile_pool(name="ps_rot2", bufs=1, space="PSUM") as ps_rot2,
            tc.tile_pool(name="ps_big", bufs=4, space="PSUM") as ps_big,
            tc.tile_pool(name="dram", bufs=1, space="DRAM") as dram,
        ):
            # ---------------- constants ----------------
            eye_sb = keep.tile([128, 128], F32)
            nc.sync.dma_start(out=eye_sb[:], in_=eye_d[:])
            eye_bf = keep.tile([128, 128], BF16)
            nc.vector.tensor_copy(out=eye_bf[:], in_=eye_sb[:])
            eye15 = keep.tile([128, 128], F32)
            nc.vector.tensor_scalar_mul(eye15[:], eye_sb[:], 1.5)
            ones_f = keep.tile([1, 128], F32)
            nc.vector.memset(ones_f[:], 1.0)
            gam_row = keep.tile([1, C], F32)
            nc.sync.dma_start(out=gam_row[:], in_=gv[:])
            bet_row = keep.tile([1, C], F32)
            nc.sync.dma_start(out=bet_row[:], in_=bv[:])

            # bf16 transposed cache: [channel, pair, position]
            XtAB = keep.tile([128, 2, NLOC], BF16)

            # --------------- pass 1: stats + transpose cache ---------------
            # Per chunk (128 positions): DMA loads cast f32->bf16 directly.
            # Per channel half: cov matmul with an embedded ones column
            # (accumulating second moments + channel sums in PSUM), and a
            # matmul against the identity producing the transposed tile.
            ps_cov = ps_acc.tile([128, 258], F32)
            ps_cov01 = ps_cov[:, 0:129]
            ps_cov23 = ps_cov[:, 129:258]

            for s in range(NSUP):
                it = inp.tile([128, SUP, 256], F32, tag="it")
                rows = xv[s * SUP * CHUNK:(s + 1) * SUP * CHUNK, :]
                rows = rows.rearrange("(c p) f -> p c f", p=128)
                nc.gpsimd.dma_start(out=it[:], in_=rows[:])

                bt = btp.tile([128, SUP, 260], BF16, tag="bt")
                nc.gpsimd.memset(bt[:, :, 128:129], 1.0)
                nc.gpsimd.memset(bt[:, :, 258:259], 1.0)
                nc.vector.tensor_scalar_mul(bt[:, :, 0:128],
                                            it[:, :, 0:128], 1.0)
                nc.vector.tensor_scalar_mul(bt[:, :, 130:258],
                                            it[:, :, 128:256], 1.0)

                for c in range(SUP):
                    k = s * SUP + c
                    first = (k == 0)
                    last = (k == NCHUNK - 1)
                    t0 = bt[:, c, 0:128]
                    t1 = bt[:, c, 130:258]
                    pot = ps_big.tile([128, 256], F32, tag="big")
                    nc.tensor.matmul(ps_cov01[:], t0, bt[:, c, 0:129],
                                     start=first, stop=last)
                    nc.tensor.matmul(pot[:, 0:128], t0, eye_bf[:],
                                     start=True, stop=True,
                                     skip_group_check=True)
                    nc.tensor.matmul(ps_cov23[:], t1, bt[:, c, 130:259],
                                     start=first, stop=last)
                    nc.tensor.matmul(pot[:, 128:256], t1, eye_bf[:],
                                     start=True, stop=True,
                                     skip_group_check=True)
                    dst = XtAB[:, :, k * CHUNK:(k + 1) * CHUNK]
                    if k % 2 == 0:
                        nc.vector.tensor_copy(out=dst, in_=pot[:])
                    else:
                        nc.scalar.copy(out=dst, in_=pot[:])

            # --------------- pack + all-reduce the stats (66KB) -----------
            # S_pk cols 0:64 = cov01 diag blocks, 64:128 = cov23 diag
            # blocks (block g on partitions 64g..), 128 = sums01, 129 = sums23.
            S_pk = keep.tile([128, 130], F32)
            nc.vector.tensor_copy(out=S_pk[0:64, 0:64],
                                  in_=ps_cov01[0:64, 0:64])
            nc.scalar.copy(out=S_pk[64:128, 0:64],
                           in_=ps_cov01[64:128, 64:128])
            nc.vector.tensor_copy(out=S_pk[0:64, 64:128],
                                  in_=ps_cov23[0:64, 0:64])
            nc.scalar.copy(out=S_pk[64:128, 64:128],
                           in_=ps_cov23[64:128, 64:128])
            nc.vector.tensor_copy(out=S_pk[:, 128:129], in_=ps_cov01[:, 128:129])
            nc.scalar.copy(out=S_pk[:, 129:130], in_=ps_cov23[:, 128:129])

            bounce_in = dram.tile([128, 130], F32)
            bounce_out = dram.tile([NCORES, 128, 130], F32)
            nc.sync.dma_start(out=bounce_in[:], in_=S_pk[:])
            nc.gpsimd.collective_compute(
                "AllGather",
                AOP.bypass,
                replica_groups=[list(range(NCORES))],
                ins=[bounce_in[:].opt()],
                outs=[bounce_out[:].opt()],
            )
            AG = keep.tile([128, NCORES, 130], F32)
            nc.sync.dma_start(
                out=AG[:], in_=bounce_out[:].rearrange("g p f -> p g f"))
            S_red = S_pk
            nc.vector.tensor_tensor(out=S_red[:], in0=AG[:, 0, :],
                                    in1=AG[:, 1, :], op=AOP.add)
            nc.gpsimd.tensor_tensor(out=AG[:, 2, :], in0=AG[:, 2, :],
                                    in1=AG[:, 3, :], op=AOP.add)
            nc.vector.tensor_tensor(out=AG[:, 4, :], in0=AG[:, 4, :],
                                    in1=AG[:, 5, :], op=AOP.add)
            nc.gpsimd.tensor_tensor(out=AG[:, 6, :], in0=AG[:, 6, :],
                                    in1=AG[:, 7, :], op=AOP.add)
            nc.vector.tensor_tensor(out=S_red[:], in0=S_red[:],
                                    in1=AG[:, 2, :], op=AOP.add)
            nc.gpsimd.tensor_tensor(out=AG[:, 4, :], in0=AG[:, 4, :],
                                    in1=AG[:, 6, :], op=AOP.add)
            nc.vector.tensor_tensor(out=S_red[:], in0=S_red[:],
                                    in1=AG[:, 4, :], op=AOP.add)

            # --------------- replicated stats assembly --------------------
            # Per pair p: PS holds [P | sigma] as (128, 256); PSb is the
            # bf16 working copy the Newton-Schulz matmuls consume.
            PS = [keep.tile([128, 256], F32, name=f"PS{p}", tag=f"PS{p}")
                  for p in range(2)]
            PSb = [keep.tile([128, 256], BF16, name=f"PSb{p}", tag=f"PSb{p}")
                   for p in range(2)]
            mu = [keep.tile([128, 1], F32, name=f"mu{p}", tag=f"mu{p}")
                  for p in range(2)]
            mu_bf = [keep.tile([128, 1], BF16, name=f"mub{p}", tag=f"mub{p}")
                     for p in range(2)]
            itr_col = [keep.tile([128, 1], F32, name=f"itr{p}", tag=f"itr{p}")
                       for p in range(2)]
            rtr_col = [keep.tile([128, 1], F32, name=f"rtr{p}", tag=f"rtr{p}")
                       for p in range(2)]
            trrow = keep.tile([1, 4], F32)

            a_coef = (1.0 - EPS) / (NGLOB - 1.0)
            b_coef = -(1.0 - EPS) * NGLOB / (NGLOB - 1.0)

            # group indicator for trace extraction: col g = 1 on partitions
            # of group-half g
            ind = keep.tile([128, 2], F32)
            nc.gpsimd.memset(ind[:], 0.0)
            nc.gpsimd.memset(ind[0:64, 0:1], 1.0)
            nc.gpsimd.memset(ind[64:128, 1:2], 1.0)
            dcol2 = keep.tile([128, 2], F32)

            for p in range(2):
                ev = nc.vector if p == 0 else nc.gpsimd
                sums = S_red[:, 128 + p:129 + p]
                ev.tensor_scalar_mul(mu[p][:], sums, 1.0 / NGLOB)
                ev.tensor_copy(out=mu_bf[p][:], in_=mu[p][:])
                # mu row via PE transpose
                ps_mur = ps_rot.tile([1, 128], F32, tag="rot")
                nc.tensor.transpose(ps_mur[:], mu[p][:], eye_sb[:])
                mur = small.tile([1, 128], F32, tag="rowtmp")
                nc.vector.tensor_copy(out=mur[:], in_=ps_mur[:])
                # mu mu^T diagonal blocks (64x64 each)
                ps_muu = ps_rot.tile([128, 64], F32, tag="rot")
                for gl in range(2):
                    nc.tensor.matmul(
                        ps_muu[64 * gl:64 * (gl + 1), 0:64],
                        mur[0:1, 64 * gl:64 * (gl + 1)],
                        mur[0:1, 64 * gl:64 * (gl + 1)],
                        start=True, stop=True,
                        tile_position=(0, 64 * gl),
                        skip_group_check=True,
                    )
                # sigma := (1-eps)*(S - N mu mu^T)/(N-1) + eps I, block-diag
                sig = PS[p][:, 128:256]
                ev.memset(sig, 0.0)
                mt = small.tile([128, 64], F32, tag=f"mt{p}", name=f"mt{p}")
                nc.vector.tensor_scalar_mul(mt[:], ps_muu[:], b_coef)
                for gl in range(2):
                    sblk = S_red[64 * gl:64 * (gl + 1), 64 * p:64 * (p + 1)]
                    nc.vector.scalar_tensor_tensor(
                        out=PS[p][64 * gl:64 * (gl + 1),
                                  128 + 64 * gl:128 + 64 * (gl + 1)],
                        in0=sblk, scalar=a_coef,
                        in1=mt[64 * gl:64 * (gl + 1), :],
                        op0=AOP.mult, op1=AOP.add,
                    )
                nc.vector.scalar_tensor_tensor(
                    out=sig, in0=eye_sb[:], scalar=EPS, in1=sig,
                    op0=AOP.mult, op1=AOP.add)
                # diag of sigma (the off-diag blocks are zero, so summing
                # the eye-masked row is the block trace contribution)
                dt_full = small.tile([128, 128], F32, tag=f"scr{p}",
                                     name=f"scr{p}")
                ev.tensor_mul(dt_full[:], sig, eye_sb[:])
                nc.vector.tensor_reduce(dcol2[:, p:p + 1], dt_full[:],
                                        axis=mybir.AxisListType.X, op=AOP.add)

            # all 4 group traces via indicator matmuls: trrow[0, 2p+gl]
            ps_tr = ps_rot.tile([1, 4], F32, tag="rot")
            for p in range(2):
                nc.tensor.matmul(ps_tr[0:1, 2 * p:2 * p + 2],
                                 dcol2[:, p:p + 1], ind[:],
                                 start=True, stop=True,
                                 skip_group_check=True)
            nc.vector.tensor_copy(out=trrow[:], in_=ps_tr[:])

            # 1/tr and 1/sqrt(tr) (+1 Newton-Raphson polish for rsqrt)
            itr_row = keep.tile([1, 4], F32)
            nc.vector.reciprocal(itr_row[:], trrow[:])
            rtr_row = keep.tile([1, 4], F32)
            sq_row = keep.tile([1, 4], F32)
            nc.scalar.activation(out=sq_row[:], in_=trrow[:], func=AFT.Sqrt)
            nc.vector.reciprocal(rtr_row[:], sq_row[:])
            nr = small.tile([1, 4], F32, tag="nr")
            nc.vector.tensor_mul(nr[:], rtr_row[:], rtr_row[:])
            nc.vector.tensor_mul(nr[:], nr[:], trrow[:])
            nc.vector.tensor_scalar(out=nr[:], in0=nr[:], scalar1=-0.5,
                                    scalar2=1.5, op0=AOP.mult, op1=AOP.add)
            nc.vector.tensor_mul(rtr_row[:], rtr_row[:], nr[:])

            # broadcast per-group scalars into per-partition columns
            for p in range(2):
                ps_itr = ps_rot.tile([128, 1], F32, tag="rot")
                ps_rtr = ps_rot.tile([128, 1], F32, tag="rot")
                for gl in range(2):
                    nc.tensor.matmul(
                        ps_itr[64 * gl:64 * (gl + 1), 0:1],
                        ones_f[0:1, 0:64],
                        itr_row[0:1, 2 * p + gl:2 * p + gl + 1],
                        start=True, stop=True, tile_position=(0, 64 * gl),
                        skip_group_check=True,
                    )
                    nc.tensor.matmul(
                        ps_rtr[64 * gl:64 * (gl + 1), 0:1],
                        ones_f[0:1, 0:64],
                        rtr_row[0:1, 2 * p + gl:2 * p + gl + 1],
                        start=True, stop=True, tile_position=(0, 64 * gl),
                        skip_group_check=True,
                    )
                nc.vector.tensor_copy(out=itr_col[p][:], in_=ps_itr[:])
                nc.vector.tensor_copy(out=rtr_col[p][:], in_=ps_rtr[:])
                # sigma /= trace ; P1 = 1.5 I - 0.5 sigma
                ev = nc.vector if p == 0 else nc.gpsimd
                sig = PS[p][:, 128:256]
                nc.vector.tensor_scalar_mul(sig, sig, itr_col[p][:])
                nc.vector.scalar_tensor_tensor(
                    out=PS[p][:, 0:128], in0=sig, scalar=-0.5, in1=eye15[:],
                    op0=AOP.mult, op1=AOP.add)
                # bf16 working copy for the NS matmuls
                ev.tensor_copy(out=PSb[p][:], in_=PS[p][:])

            # Newton-Schulz iterations 2..5 in bf16:
            #   [P^2 | P sigma] = P @ [P | sigma];  P' = 1.5 P - 0.5 P^2 (P sigma)
            for _ in range(ITER_NUM - 1):
                for p in range(2):
                    ps1 = ps_rot.tile([128, 256], F32, tag="rot")
                    nc.tensor.matmul(ps1[:], PSb[p][:, 0:128], PSb[p][:, 0:256],
                                     start=True, stop=True)
                    tmp = small.tile([128, 256], BF16, tag=f"nstmp{p}",
                                     name=f"nstmp{p}")
                    if p == 0:
                        nc.vector.tensor_copy(out=tmp[:], in_=ps1[:])
                    else:
                        nc.scalar.copy(out=tmp[:], in_=ps1[:])
                    # 1.5*P computed in parallel to the 2nd matmul
                    tP = small.tile([128, 128], BF16, tag=f"tP{p}",
                                    name=f"tP{p}")
                    if p == 0:
                        nc.scalar.activation(out=tP[:], in_=PSb[p][:, 0:128],
                                             func=AFT.Copy, scale=1.5)
                    else:
                        nc.vector.tensor_scalar_mul(tP[:], PSb[p][:, 0:128],
                                                    1.5)
                    ps2 = ps_rot2.tile([128, 128], F32, tag="rot2")
                    nc.tensor.matmul(ps2[:], tmp[:, 0:128], tmp[:, 128:256],
                                     start=True, stop=True)
                    nc.vector.scalar_tensor_tensor(
                        out=PSb[p][:, 0:128], in0=ps2[:], scalar=-0.5,
                        in1=tP[:], op0=AOP.mult, op1=AOP.add)

            # W_g = (P / sqrt(tr)) * gamma (channel-major, bf16 stationary);
            # bias col = beta - W_g^T mu
            Wbf = [keep.tile([128, 128], BF16, name=f"Wbf{p}", tag=f"Wbf{p}")
                   for p in range(2)]
            bcol = [keep.tile([128, 1], F32, name=f"bcol{p}", tag=f"bcol{p}")
                    for p in range(2)]
            for p in range(2):
                wmf = small.tile([128, 128], F32, tag="wmf")
                nc.vector.tensor_scalar_mul(wmf[:], PSb[p][:, 0:128],
                                            rtr_col[p][:])
                ps_g = ps_rot.tile([128, 128], F32, tag="rot")
                nc.tensor.matmul(ps_g[:], ones_f[0:1, 0:128],
                                 gam_row[0:1, 128 * p:128 * (p + 1)],
                                 start=True, stop=True)
                nc.vector.tensor_tensor(out=Wbf[p][:], in0=wmf[:],
                                        in1=ps_g[:], op=AOP.mult)
                # beta column: transpose the beta row segment (K=1 matmul)
                ps_bt = ps_rot.tile([128, 1], F32, tag="rot")
                nc.tensor.matmul(ps_bt[:], bet_row[0:1, 128 * p:128 * (p + 1)],
                                 ones_f[0:1, 0:1], start=True, stop=True)
                bet_col = small.tile([128, 1], F32, tag="betc")
                nc.vector.tensor_copy(out=bet_col[:], in_=ps_bt[:])
                ps_b = ps_rot.tile([128, 1], F32, tag="rot")
                nc.tensor.matmul(ps_b[:], Wbf[p][:], mu_bf[p][:],
                                 start=True, stop=True)
                nc.vector.scalar_tensor_tensor(
                    out=bcol[p][:], in0=ps_b[:], scalar=-1.0, in1=bet_col[:],
                    op0=AOP.mult, op1=AOP.add)

            # --------------- pass 2: whiten, channel-major ---------------
            # out[co, pos] = W_g^T Xt + bias_col; W stationary, Xt moving.
            eng = 0
            for p in range(2):
                for w in range(NWIN):
                    ot = outp.tile([128, POSW], BF16, tag="ot")
                    for j in range(NMM):
                        lo = w * POSW + j * MMW
                        po = ps_big.tile([128, MMW], F32, tag="big")
                        nc.tensor.matmul(po[:], Wbf[p][:],
                                         XtAB[:, p, lo:lo + MMW],
                                         start=True, stop=True)
                        dst = ot[:, j * MMW:(j + 1) * MMW]
                        if eng == 0:
                            nc.vector.tensor_scalar_add(dst, po[:],
                                                        bcol[p][:])
                        else:
                            nc.scalar.activation(out=dst, in_=po[:],
                                                 func=AFT.Identity,
                                                 bias=bcol[p][:], scale=1.0)
                        eng = (eng + 1) % 2
                    nc.sync.dma_start(
                        out=y_d[p, :, w * POSW:(w + 1) * POSW], in_=ot[:])

    nc.finalize()
    return nc


_NC_CACHE = None


def _get_nc():
    global _NC_CACHE
    if _NC_CACHE is None:
        _NC_CACHE = build_bass()
    return _NC_CACHE


def make_in_maps(x, gamma, beta):
    x = np.ascontiguousarray(np.asarray(x, dtype=np.float32))
    gamma = np.asarray(gamma, dtype=np.float32)
    beta = np.asarray(beta, dtype=np.float32)
    eye = np.eye(128, dtype=np.float32)
    maps = []
    for i in range(NCORES):
        maps.append({
            "x": np.ascontiguousarray(x[i * BLOC:(i + 1) * BLOC]),
            "gamma": gamma,
            "beta": beta,
            "eye": eye,
        })
    return maps


def unshard(results):
    parts = []
    for i in range(NCORES):
        o = np.asarray(results[i]["out"]).astype(np.float32)
        # (2, 128, NLOC) channel-major -> (BLOC, H, W, C)
        parts.append(o.reshape(C, BLOC, H, W).transpose(1, 2, 3, 0))
    return np.ascontiguousarray(np.concatenate(parts, axis=0))


def kernel(x, gamma, beta):
    nc = _get_nc()
    in_maps = make_in_maps(x, gamma, beta)
    res = run_bass_kernel_spmd(nc, in_maps, core_ids=list(range(NCORES)))
    return unshard(res.results)


if __name__ == "__main__":
    nc = build_bass()
    print("graph built OK")


# revision 36
# speedup vs baseline: 1.0432x; 1.0432x over previous
"""Trainium2 Bass kernel: DecorrelationNormalization (IterNorm whitening).

Input  x: (64, 56, 56, 256) f32, gamma/beta: (1,1,1,256) f32.
Sharding: data-parallel over batch across 8 NeuronCores (8 batches/core).
Per-group (4 groups of 64 channels) covariance stats are computed locally
as uncentered second moments + channel sums, packed into 66KB, all-reduced,
the tiny Newton-Schulz iteration is replicated on every core (bf16 matmuls),
and the whitening is applied channel-major with W stationary, writing bf16
output that the host transposes/upcasts.
"""

import sys

for p in ("/opt/trn_rl_repo", "/opt/pypackages"):
    if p not in sys.path:
        sys.path.append(p)

import numpy as np

import concourse.bass as bass
import concourse.bacc as bacc
import concourse.tile as tile
from concourse import mybir
from concourse.bass_utils import run_bass_kernel_spmd
F32 = mybir.dt.float32
BF16 = mybir.dt.bfloat16

# Problem constants (hardcoded per spec).
B, H, W, C = 64, 56, 56, 256
NCORES = 8
BLOC = B // NCORES                    # 8 batches per core
NLOC = BLOC * H * W                   # 25088 positions per core
NGLOB = B * H * W                     # 200704 positions globally
CHUNK = 128                           # positions per chunk (partition dim)
NCHUNK = NLOC // CHUNK                # 196
SUP = 14                              # chunks per super-tile (DMA batch)
NSUP = NCHUNK // SUP                  # 14, exact
POSW = 3584                           # pass-2 output window (positions)
NWIN = NLOC // POSW                   # 7, exact
MMW = 512                             # pass-2 matmul moving width
NMM = POSW // MMW                     # 7
EPS = 1e-5
ITER_NUM = 5

AOP = mybir.AluOpType
AFT = mybir.ActivationFunctionType


def build_bass() -> bass.Bass:
    nc = bacc.Bacc(None, num_devices=NCORES)

    x_d = nc.declare_dram_parameter("x", [BLOC, H, W, C], F32, isOutput=False)
    g_d = nc.declare_dram_parameter("gamma", [1, 1, 1, C], F32, isOutput=False)
    b_d = nc.declare_dram_parameter("beta", [1, 1, 1, C], F32, isOutput=False)
    eye_d = nc.declare_dram_parameter("eye", [128, 128], F32, isOutput=False)
    y_d = nc.declare_dram_parameter("out", [2, 128, NLOC], BF16, isOutput=True)

    xv = x_d[:].flatten_outer_dims()     # (25088, 256)
    gv = g_d[:].flatten_outer_dims()     # (1, 256)
    bv = b_d[:].flatten_outer_dims()     # (1, 256)

    with tile.TileContext(nc) as tc:
        with (
            tc.tile_pool(name="keep", bufs=1) as keep,
            tc.tile_pool(name="inp", bufs=3) as inp,
            tc.tile_pool(name="btp", bufs=3) as btp,
            tc.tile_pool(name="outp", bufs=3) as outp,
            tc.tile_pool(name="small", bufs=1) as small,
            tc.tile_pool(name="ps_acc", bufs=1, space="PSUM") as ps_acc,
            tc.tile_pool(name="ps_rot", bufs=2, space="PSUM") as ps_rot,
            tc.tile_pool(name="ps_rot2", bufs=1, space="PSUM") as ps_rot2,
            tc.tile_pool(name="ps_big", bufs=4, space="PSUM") as ps_big,
            tc.tile_pool(name="dram", bufs=1, space="DRAM") as dram,
        ):
            # ---------------- constants ----------------
            eye_sb = keep.tile([128, 128], F32)
            nc.sync.dma_start(out=eye_sb[:], in_=eye_d[:])
            eye_bf = keep.tile([128, 128], BF16)
            nc.vector.tensor_copy(out=eye_bf[:], in_=eye_sb[:])
            eye15 = keep.tile([128, 128], F32)
            nc.vector.tensor_scalar_mul(eye15[:], eye_sb[:], 1.5)
            ones_f = keep.tile([1, 128], F32)
            nc.vector.memset(ones_f[:], 1.0)
            gam_row = keep.tile([1, C], F32)
            nc.sync.dma_start(out=gam_row[:], in_=gv[:])
            bet_row = keep.tile([1, C], F32)
            nc.sync.dma_start(out=bet_row[:], in_=bv[:])

            # bf16 transposed cache: [channel, pair, position]
            XtAB = keep.tile([128, 2, NLOC], BF16)

            # --------------- pass 1: stats + transpose cache ---------------
            # Per chunk (128 positions): DMA loads cast f32->bf16 directly.
            # Per channel half: cov matmul with an embedded ones column
            # (accumulating second moments + channel sums in PSUM), and a
            # matmul against the identity producing the transposed tile.
            ps_cov = ps_acc.tile([128, 258], F32)
            ps_cov01 = ps_cov[:, 0:129]
            ps_cov23 = ps_cov[:, 129:258]
            deferred = []

            for s in range(NSUP):
                it = inp.tile([128, SUP, 256], F32, tag="it")
                rows = xv[s * SUP * CHUNK:(s + 1) * SUP * CHUNK, :]
                rows = rows.rearrange("(c p) f -> p c f", p=128)
                nc.gpsimd.dma_start(out=it[:], in_=rows[:])

                bt = btp.tile([128, SUP, 260], BF16, tag="bt")
                nc.gpsimd.memset(bt[:, :, 128:129], 1.0)
                nc.gpsimd.memset(bt[:, :, 258:259], 1.0)
                nc.vector.tensor_scalar_mul(bt[:, :, 0:128],
                                            it[:, :, 0:128], 1.0)
                nc.vector.tensor_scalar_mul(bt[:, :, 130:258],
                                            it[:, :, 128:256], 1.0)

                for c in range(SUP):
                    k = s * SUP + c
                    first = (k == 0)
                    last = (k == NCHUNK - 1)
                    t0 = bt[:, c, 0:128]
                    t1 = bt[:, c, 130:258]
                    nc.tensor.matmul(ps_cov01[:], t0, bt[:, c, 0:129],
                                     start=first, stop=last)
                    if s >= NSUP - 3:
                        # Defer this chunk's transposes: keeps the tail of
                        # the cov stream unobstructed on the PE queue so the
                        # all-gather launches ~9us earlier; the transposes
                        # (re-loading weights) hide inside the collective.
                        nc.tensor.matmul(ps_cov23[:], t1, bt[:, c, 130:259],
                                         start=first, stop=last)
                        deferred.append((bt, c, k))
                        continue
                    pot = ps_big.tile([128, 256], F32, tag="big")
                    nc.tensor.matmul(pot[:, 0:128], t0, eye_bf[:],
                                     start=True, stop=True,
                                     skip_group_check=True)
                    nc.tensor.matmul(ps_cov23[:], t1, bt[:, c, 130:259],
                                     start=first, stop=last)
                    nc.tensor.matmul(pot[:, 128:256], t1, eye_bf[:],
                                     start=True, stop=True,
                                     skip_group_check=True)
                    dst = XtAB[:, :, k * CHUNK:(k + 1) * CHUNK]
                    if k % 2 == 0:
                        nc.vector.tensor_copy(out=dst, in_=pot[:])
                    else:
                        nc.scalar.copy(out=dst, in_=pot[:])

            # --------------- pack + all-reduce the stats (66KB) -----------
            # S_pk cols 0:64 = cov01 diag blocks, 64:128 = cov23 diag
            # blocks (block g on partitions 64g..), 128 = sums01, 129 = sums23.
            S_pk = keep.tile([128, 130], F32)
            nc.vector.tensor_copy(out=S_pk[0:64, 0:64],
                                  in_=ps_cov01[0:64, 0:64])
            nc.scalar.copy(out=S_pk[64:128, 0:64],
                           in_=ps_cov01[64:128, 64:128])
            nc.vector.tensor_copy(out=S_pk[0:64, 64:128],
                                  in_=ps_cov23[0:64, 0:64])
            nc.scalar.copy(out=S_pk[64:128, 64:128],
                           in_=ps_cov23[64:128, 64:128])
            nc.vector.tensor_copy(out=S_pk[:, 128:129], in_=ps_cov01[:, 128:129])
            nc.scalar.copy(out=S_pk[:, 129:130], in_=ps_cov23[:, 128:129])

            bounce_in = dram.tile([128, 130], F32)
            bounce_out = dram.tile([NCORES, 128, 130], F32)
            nc.sync.dma_start(out=bounce_in[:], in_=S_pk[:])
            nc.gpsimd.collective_compute(
                "AllGather",
                AOP.bypass,
                replica_groups=[list(range(NCORES))],
                ins=[bounce_in[:].opt()],
                outs=[bounce_out[:].opt()],
            )
            # deferred tail transposes run while the collective is in flight
            for bt_d, c, k in deferred:
                t0 = bt_d[:, c, 0:128]
                t1 = bt_d[:, c, 130:258]
                pot = ps_big.tile([128, 256], F32, tag="big")
                nc.tensor.matmul(pot[:, 0:128], t0, eye_bf[:],
                                 start=True, stop=True,
                                 skip_group_check=True)
                nc.tensor.matmul(pot[:, 128:256], t1, eye_bf[:],
                                 start=True, stop=True,
                                 skip_group_check=True)
                dst = XtAB[:, :, k * CHUNK:(k + 1) * CHUNK]
                if k % 2 == 0:
                    nc.vector.tensor_copy(out=dst, in_=pot[:])
                else:
                    nc.scalar.copy(out=dst, in_=pot[:])

            AG = keep.tile([128, NCORES, 130], F32)
            nc.sync.dma_start(
                out=AG[:], in_=bounce_out[:].rearrange("g p f -> p g f"))
            S_red = S_pk
            nc.vector.tensor_tensor(out=S_red[:], in0=AG[:, 0, :],
                                    in1=AG[:, 1, :], op=AOP.add)
            nc.gpsimd.tensor_tensor(out=AG[:, 2, :], in0=AG[:, 2, :],
                                    in1=AG[:, 3, :], op=AOP.add)
            nc.vector.tensor_tensor(out=AG[:, 4, :], in0=AG[:, 4, :],
                                    in1=AG[:, 5, :], op=AOP.add)
            nc.gpsimd.tensor_tensor(out=AG[:, 6, :], in0=AG[:, 6, :],
                                    in1=AG[:, 7, :], op=AOP.add)
            nc.vector.tensor_tensor(out=S_red[:], in0=S_red[:],
                                    in1=AG[:, 2, :], op=AOP.add)
            nc.gpsimd.tensor_tensor(out=AG[:, 4, :], in0=AG[:, 4, :],
                                    in1=AG[:, 6, :], op=AOP.add)
            nc.vector.tensor_tensor(out=S_red[:], in0=S_red[:],
                                    in1=AG[:, 4, :], op=AOP.add)

            # --------------- replicated stats assembly --------------------
            # Per pair p: PS holds [P | sigma] as (128, 256); PSb is the
            # bf16 working copy the Newton-Schulz matmuls consume.
            PS = [keep.tile([128, 256], F32, name=f"PS{p}", tag=f"PS{p}")
                  for p in range(2)]
            PSb = [keep.tile([128, 256], BF16, name=f"PSb{p}", tag=f"PSb{p}")
                   for p in range(2)]
            mu = [keep.tile([128, 1], F32, name=f"mu{p}", tag=f"mu{p}")
                  for p in range(2)]
            mu_bf = [keep.tile([128, 1], BF16, name=f"mub{p}", tag=f"mub{p}")
                     for p in range(2)]
            itr_col = [keep.tile([128, 1], F32, name=f"itr{p}", tag=f"itr{p}")
                       for p in range(2)]
            rtr_col = [keep.tile([128, 1], F32, name=f"rtr{p}", tag=f"rtr{p}")
                       for p in range(2)]
            trrow = keep.tile([1, 4], F32)

            a_coef = (1.0 - EPS) / (NGLOB - 1.0)
            b_coef = -(1.0 - EPS) * NGLOB / (NGLOB - 1.0)

            # group indicator for trace extraction: col g = 1 on partitions
            # of group-half g
            ind = keep.tile([128, 2], F32)
            nc.gpsimd.memset(ind[:], 0.0)
            nc.gpsimd.memset(ind[0:64, 0:1], 1.0)
            nc.gpsimd.memset(ind[64:128, 1:2], 1.0)
            dcol2 = keep.tile([128, 2], F32)

            for p in range(2):
                ev = nc.vector if p == 0 else nc.gpsimd
                sums = S_red[:, 128 + p:129 + p]
                ev.tensor_scalar_mul(mu[p][:], sums, 1.0 / NGLOB)
                ev.tensor_copy(out=mu_bf[p][:], in_=mu[p][:])
                # mu row via PE transpose
                ps_mur = ps_rot.tile([1, 128], F32, tag="rot")
                nc.tensor.transpose(ps_mur[:], mu[p][:], eye_sb[:])
                mur = small.tile([1, 128], F32, tag="rowtmp")
                nc.vector.tensor_copy(out=mur[:], in_=ps_mur[:])
                # mu mu^T diagonal blocks (64x64 each)
                ps_muu = ps_rot.tile([128, 64], F32, tag="rot")
                for gl in range(2):
                    nc.tensor.matmul(
                        ps_muu[64 * gl:64 * (gl + 1), 0:64],
                        mur[0:1, 64 * gl:64 * (gl + 1)],
                        mur[0:1, 64 * gl:64 * (gl + 1)],
                        start=True, stop=True,
                        tile_position=(0, 64 * gl),
                        skip_group_check=True,
                    )
                # sigma := (1-eps)*(S - N mu mu^T)/(N-1) + eps I, block-diag
                sig = PS[p][:, 128:256]
                ev.memset(sig, 0.0)
                mt = small.tile([128, 64], F32, tag=f"mt{p}", name=f"mt{p}")
                nc.vector.tensor_scalar_mul(mt[:], ps_muu[:], b_coef)
                for gl in range(2):
                    sblk = S_red[64 * gl:64 * (gl + 1), 64 * p:64 * (p + 1)]
                    nc.vector.scalar_tensor_tensor(
                        out=PS[p][64 * gl:64 * (gl + 1),
                                  128 + 64 * gl:128 + 64 * (gl + 1)],
                        in0=sblk, scalar=a_coef,
                        in1=mt[64 * gl:64 * (gl + 1), :],
                        op0=AOP.mult, op1=AOP.add,
                    )
                nc.vector.scalar_tensor_tensor(
                    out=sig, in0=eye_sb[:], scalar=EPS, in1=sig,
                    op0=AOP.mult, op1=AOP.add)
                # diag of sigma (the off-diag blocks are zero, so summing
                # the eye-masked row is the block trace contribution)
                dt_full = small.tile([128, 128], F32, tag=f"scr{p}",
                                     name=f"scr{p}")
                ev.tensor_mul(dt_full[:], sig, eye_sb[:])
                nc.vector.tensor_reduce(dcol2[:, p:p + 1], dt_full[:],
                                        axis=mybir.AxisListType.X, op=AOP.add)

            # all 4 group traces via indicator matmuls: trrow[0, 2p+gl]
            ps_tr = ps_rot.tile([1, 4], F32, tag="rot")
            for p in range(2):
                nc.tensor.matmul(ps_tr[0:1, 2 * p:2 * p + 2],
                                 dcol2[:, p:p + 1], ind[:],
                                 start=True, stop=True,
                                 skip_group_check=True)
            nc.vector.tensor_copy(out=trrow[:], in_=ps_tr[:])

            # 1/tr and 1/sqrt(tr) (+1 Newton-Raphson polish for rsqrt)
            itr_row = keep.tile([1, 4], F32)
            nc.vector.reciprocal(itr_row[:], trrow[:])
            rtr_row = keep.tile([1, 4], F32)
            sq_row = keep.tile([1, 4], F32)
            nc.scalar.activation(out=sq_row[:], in_=trrow[:], func=AFT.Sqrt)
            nc.vector.reciprocal(rtr_row[:], sq_row[:])
            nr = small.tile([1, 4], F32, tag="nr")
            nc.vector.tensor_mul(nr[:], rtr_row[:], rtr_row[:])
            nc.vector.tensor_mul(nr[:], nr[:], trrow[:])
            nc.vector.tensor_scalar(out=nr[:], in0=nr[:], scalar1=-0.5,
                                    scalar2=1.5, op0=AOP.mult, op1=AOP.add)
            nc.vector.tensor_mul(rtr_row[:], rtr_row[:], nr[:])

            # broadcast per-group scalars into per-partition columns
            for p in range(2):
                ps_itr = ps_rot.tile([128, 1], F32, tag="rot")
                ps_rtr = ps_rot.tile([128, 1], F32, tag="rot")
                for gl in range(2):
                    nc.tensor.matmul(
                        ps_itr[64 * gl:64 * (gl + 1), 0:1],
                        ones_f[0:1, 0:64],
                        itr_row[0:1, 2 * p + gl:2 * p + gl + 1],
                        start=True, stop=True, tile_position=(0, 64 * gl),
                        skip_group_check=True,
                    )
                    nc.tensor.matmul(
                        ps_rtr[64 * gl:64 * (gl + 1), 0:1],
                        ones_f[0:1, 0:64],
                        rtr_row[0:1, 2 * p + gl:2 * p + gl + 1],
                        start=True, stop=True, tile_position=(0, 64 * gl),
                        skip_group_check=True,
                    )
                nc.vector.tensor_copy(out=itr_col[p][:], in_=ps_itr[:])
                nc.vector.tensor_copy(out=rtr_col[p][:], in_=ps_rtr[:])
                # sigma /= trace ; P1 = 1.5 I - 0.5 sigma
                ev = nc.vector if p == 0 else nc.gpsimd
                sig = PS[p][:, 128:256]
                nc.vector.tensor_scalar_mul(sig, sig, itr_col[p][:])
                nc.vector.scalar_tensor_tensor(
                    out=PS[p][:, 0:128], in0=sig, scalar=-0.5, in1=eye15[:],
                    op0=AOP.mult, op1=AOP.add)
                # bf16 working copy for the NS matmuls
                ev.tensor_copy(out=PSb[p][:], in_=PS[p][:])

            # Newton-Schulz iterations 2..5 in bf16:
            #   [P^2 | P sigma] = P @ [P | sigma];  P' = 1.5 P - 0.5 P^2 (P sigma)
            for _ in range(ITER_NUM - 1):
                for p in range(2):
                    ps1 = ps_rot.tile([128, 256], F32, tag="rot")
                    nc.tensor.matmul(ps1[:], PSb[p][:, 0:128], PSb[p][:, 0:256],
                                     start=True, stop=True)
                    tmp = small.tile([128, 256], BF16, tag=f"nstmp{p}",
                                     name=f"nstmp{p}")
                    if p == 0:
                        nc.vector.tensor_copy(out=tmp[:], in_=ps1[:])
                    else:
                        nc.scalar.copy(out=tmp[:], in_=ps1[:])
                    # 1.5*P computed in parallel to the 2nd matmul
                    tP = small.tile([128, 128], BF16, tag=f"tP{p}",
                                    name=f"tP{p}")
                    if p == 0:
                        nc.scalar.activation(out=tP[:], in_=PSb[p][:, 0:128],
                                             func=AFT.Copy, scale=1.5)
                    else:
                        nc.vector.tensor_scalar_mul(tP[:], PSb[p][:, 0:128],
                                                    1.5)
                    ps2 = ps_rot2.tile([128, 128], F32, tag="rot2")
                    nc.tensor.matmul(ps2[:], tmp[:, 0:128], tmp[:, 128:256],
                                     start=True, stop=True)
                    nc.vector.scalar_tensor_tensor(
                        out=PSb[p][:, 0:128], in0=ps2[:], scalar=-0.5,
                        in1=tP[:], op0=AOP.mult, op1=AOP.add)

            # W_g = (P / sqrt(tr)) * gamma (channel-major, bf16 stationary);
            # bias col = beta - W_g^T mu
            Wbf = [keep.tile([128, 128], BF16, name=f"Wbf{p}", tag=f"Wbf{p}")
                   for p in range(2)]
            bcol = [keep.tile([128, 1], F32, name=f"bcol{p}", tag=f"bcol{p}")
                    for p in range(2)]
            for p in range(2):
                wmf = small.tile([128, 128], F32, tag="wmf")
                nc.vector.tensor_scalar_mul(wmf[:], PSb[p][:, 0:128],
                                            rtr_col[p][:])
                ps_g = ps_rot.tile([128, 128], F32, tag="rot")
                nc.tensor.matmul(ps_g[:], ones_f[0:1, 0:128],
                                 gam_row[0:1, 128 * p:128 * (p + 1)],
                                 start=True, stop=True)
                nc.vector.tensor_tensor(out=Wbf[p][:], in0=wmf[:],
                                        in1=ps_g[:], op=AOP.mult)
                # beta column: transpose the beta row segment (K=1 matmul)
                ps_bt = ps_rot.tile([128, 1], F32, tag="rot")
                nc.tensor.matmul(ps_bt[:], bet_row[0:1, 128 * p:128 * (p + 1)],
                                 ones_f[0:1, 0:1], start=True, stop=True)
                bet_col = small.tile([128, 1], F32, tag="betc")
                nc.vector.tensor_copy(out=bet_col[:], in_=ps_bt[:])
                ps_b = ps_rot.tile([128, 1], F32, tag="rot")
                nc.tensor.matmul(ps_b[:], Wbf[p][:], mu_bf[p][:],
                                 start=True, stop=True)
                nc.vector.scalar_tensor_tensor(
                    out=bcol[p][:], in0=ps_b[:], scalar=-1.0, in1=bet_col[:],
                    op0=AOP.mult, op1=AOP.add)

            # --------------- pass 2: whiten, channel-major ---------------
            # out[co, pos] = W_g^T Xt + bias_col; W stationary, Xt moving.
            eng = 0
            for p in range(2):
                for w in range(NWIN):
                    ot = outp.tile([128, POSW], BF16, tag="ot")
                    for j in range(NMM):
                        lo = w * POSW + j * MMW
                        po = ps_big.tile([128, MMW], F32, tag="big")
                        nc.tensor.matmul(po[:], Wbf[p][:],
                                         XtAB[:, p, lo:lo + MMW],
                                         start=True, stop=True)
                        dst = ot[:, j * MMW:(j + 1) * MMW]
                        if eng == 0:
                            nc.vector.tensor_scalar_add(dst, po[:],
                                                        bcol[p][:])
                        else:
                            nc.scalar.activation(out=dst, in_=po[:],
                                                 func=AFT.Identity,
                                                 bias=bcol[p][:], scale=1.0)
                        eng = (eng + 1) % 2
                    nc.sync.dma_start(
                        out=y_d[p, :, w * POSW:(w + 1) * POSW], in_=ot[:])

    nc.finalize()
    return nc


_NC_CACHE = None


def _get_nc():
    global _NC_CACHE
    if _NC_CACHE is None:
        _NC_CACHE = build_bass()
    return _NC_CACHE


def make_in_maps(x, gamma, beta):
    x = np.ascontiguousarray(np.asarray(x, dtype=np.float32))
    gamma = np.asarray(gamma, dtype=np.float32)
    beta = np.asarray(beta, dtype=np.float32)
    eye = np.eye(128, dtype=np.float32)
    maps = []
    for i in range(NCORES):
        maps.append({
            "x": np.ascontiguousarray(x[i * BLOC:(i + 1) * BLOC]),
            "gamma": gamma,
            "beta": beta,
            "eye": eye,
        })
    return maps


def unshard(results):
    parts = []
    for i in range(NCORES):
        o = np.asarray(results[i]["out"]).astype(np.float32)
        # (2, 128, NLOC) channel-major -> (BLOC, H, W, C)
        parts.append(o.reshape(C, BLOC, H, W).transpose(1, 2, 3, 0))
    return np.ascontiguousarray(np.concatenate(parts, axis=0))


def kernel(x, gamma, beta):
    nc = _get_nc()
    in_maps = make_in_maps(x, gamma, beta)
    res = run_bass_kernel_spmd(nc, in_maps, core_ids=list(range(NCORES)))
    return unshard(res.results)


if __name__ == "__main__":
    nc = build_bass()
    print("graph built OK")
